# revision 10
# baseline (speedup 1.0000x reference)
"""Self-contained TRN2 Bass kernel for NeuralFSM message passing.

kernel(s0, edge_index, T) -> [100000, 8] float32, running 20 FSM iterations
on 8 NeuronCores via concourse/bass (SPMD, node-partitioned).

Algorithm: states are one-hot; threshold(segment_sum) == bitwise OR of
neighbor state bits. Per node keep a byte b = 1<<state packed 4-per-uint32
in an SBUF-resident table W replicated across partitions. Edge slots are
split into 4 classes by the source node's byte lane, so the lane shift is a
compile-time constant per class (no per-slot shift metadata): per iteration,
ap_gather the static class streams, OR-reduce per destination, fold the four
class partials with constant shifts, look up a merged 256x8 FSM transition
table ((1<<next)<<16 | next+1) with one gather, repack the words, AllGather
the 12.5KB word slab across the 8 cores, and re-broadcast into W.
"""
import os
import sys

import numpy as np

for _p in ("/opt/trn_rl_repo", "/root/.axon_site/_ro/trn_rl_repo", "/root/.axon_site"):
    if os.path.isdir(_p) and _p not in sys.path:
        sys.path.append(_p)

N_REAL = 100000
S = 8
NC = 8
P = 128
RPC = 16
GROUPS = 8                        # gpsimd cores (16-partition groups) per NC
JROWS = 98
NODES_CORE = RPC * JROWS          # 1568 dst nodes per group
NDST_NC = P * JROWS               # 12544
NTOT = NC * NDST_NC               # 100352
WORDS_CORE = NODES_CORE // 4      # 392
WORDS_NC = WORDS_CORE * GROUPS    # 3136
NWORDS = 1 + NC * WORDS_NC        # 25089
T2SIZE = 2048
ITERS = 20
CHUNK_BUDGET = 3584

LAST_EXEC_NS = None
LAST_TRACE_PATH = None


class _Layout:
    def __init__(self, edge_index):
        src_all = edge_index[0].astype(np.int64)
        dst_all = edge_index[1].astype(np.int64)
        deg = np.bincount(dst_all, minlength=N_REAL)
        L_node = np.maximum(1, -(-deg // 4)) * 4
        order = np.argsort(-L_node, kind="stable")
        node_of_z = np.full(NTOT, -1, dtype=np.int64)
        node_of_z[:N_REAL] = order
        z = np.arange(NTOT)
        j_of_z = z // (NC * P)
        nc_of_z = (z % (NC * P)) // P
        p_of_z = z % P
        z_of_node = np.full(N_REAL, -1, dtype=np.int64)
        z_of_node[order] = np.arange(N_REAL)

        # ---- greedy lane balancing (per-destination class counts) ----
        # cell = (j, nc, group): 16 placement slots, exactly 4 per lane
        NCELLS = JROWS * NC * GROUPS
        cell_of_z = (j_of_z * NC + nc_of_z) * GROUPS + p_of_z // RPC
        cell_of_node = cell_of_z[z_of_node]
        eo = np.argsort(src_all, kind="stable")
        dst_by_src = dst_all[eo]
        s_sorted = src_all[eo]
        starts = np.searchsorted(s_sorted, np.arange(N_REAL))
        ends = np.searchsorted(s_sorted, np.arange(N_REAL) + 1)
        outdeg = ends - starts
        row_of_node = j_of_z[z_of_node]
        degz = np.zeros(NTOT, dtype=np.int64)
        degz[z_of_node] = deg
        rowmax = degz.reshape(JROWS, NC * P).max(axis=1)
        Lfloor = np.maximum(1, -(-rowmax // 4))
        Lrl_live = np.tile(Lfloor[:, None], (1, 4)).astype(np.int64)
        cnt = np.zeros((N_REAL, 4), dtype=np.int32)
        quota = np.zeros((NCELLS, 4), dtype=np.int16)
        lane_of_node = np.zeros(N_REAL, dtype=np.int8)
        big = np.int64(1) << 40
        for u in np.argsort(-outdeg, kind="stable"):
            d = dst_by_src[starts[u]:ends[u]]
            c = cnt[d]
            rows = row_of_node[d]
            inc = np.maximum(0, c + 1 - Lrl_live[rows]).sum(axis=0, dtype=np.int64)
            ssum = c.sum(axis=0, dtype=np.int64)
            score = inc * (np.int64(1) << 22) + ssum
            q = quota[cell_of_node[u]]
            score = np.where(q < 4, score, big)
            l = int(np.argmin(score))
            lane_of_node[u] = l
            np.add.at(cnt, (d, np.full(len(d), l)), 1)
            np.maximum.at(Lrl_live, (rows, np.full(len(d), l)), cnt[d, l])
            quota[cell_of_node[u], l] += 1

        # ---- re-place nodes within each cell so r%4 == lane ----
        new_node_of_z = np.full(NTOT, -1, dtype=np.int64)
        z_by_cell = np.argsort(cell_of_z, kind="stable").reshape(NCELLS, RPC)
        # z slots within a cell sorted by r (p%16): cell_of_z ordering is by z,
        # and within a cell z increases with p, i.e. with r.
        for cid in range(NCELLS):
            zs = z_by_cell[cid]
            nodes = node_of_z[zs]
            nodes = nodes[nodes >= 0]
            used = np.zeros(RPC, dtype=bool)
            lane_slots = [[l, l + 4, l + 8, l + 12] for l in range(4)]
            ptr = [0, 0, 0, 0]
            for u in nodes:
                l = lane_of_node[u]
                r = lane_slots[l][ptr[l]]
                ptr[l] += 1
                new_node_of_z[zs[r]] = u
                used[r] = True
        node_of_z = new_node_of_z
        z_of_node = np.full(N_REAL, -1, dtype=np.int64)
        valid = node_of_z >= 0
        z_of_node[node_of_z[valid]] = z[valid]

        self.j_of_z = j_of_z
        self.nc_of_z = nc_of_z
        self.p_of_z = p_of_z
        self.node_of_z = node_of_z
        self.z_of_node = z_of_node

        g_of_z = p_of_z // RPC
        r_of_z = p_of_z % RPC
        self.word_of_z = 1 + nc_of_z * WORDS_NC + g_of_z * WORDS_CORE \
            + 4 * j_of_z + r_of_z // 4
        self.lane_of_z = r_of_z % 4

        # ---- per-row per-class padded lengths and chunks ----
        cnt_z = np.zeros((NTOT, 4), dtype=np.int64)
        cnt_z[z_of_node[node_of_z[valid]]] = 0  # init
        cnt_z[z_of_node] = cnt  # index by node's z
        # L multiple of 4 keeps every chunk's num_idxs a multiple of 64
        # (the gather ucode reads wrapped int16 indices in 8B groups)
        Lrl = -(-np.maximum(cnt_z.reshape(JROWS, NC * P, 4).max(axis=1), 1)
                // 4) * 4

        chunks = []   # (cls, j0, jr, L, to)
        to = 0
        for l in range(4):
            Lj = Lrl[:, l]
            j = 0
            while j < JROWS:
                j2 = j
                curmax = 0
                while j2 < JROWS:
                    m2 = max(curmax, int(Lj[j2]))
                    if RPC * (j2 - j + 1) * m2 > CHUNK_BUDGET and j2 > j:
                        break
                    curmax = m2
                    j2 += 1
                    if RPC * m2 > CHUNK_BUDGET:
                        break
                chunks.append((l, j, j2 - j, curmax, to))
                to += RPC * (j2 - j) * curmax
                j = j2
        self.chunks = chunks
        self.slots_per_core = to
        assert max(RPC * jr * L for (_, _, jr, L, _) in chunks) <= CHUNK_BUDGET

        # per (class, row): L and stream base offset
        base_lj = np.zeros((4, JROWS), dtype=np.int64)
        L_lj = np.zeros((4, JROWS), dtype=np.int64)
        for (l, j0, jr, L, toff) in chunks:
            for jj in range(jr):
                base_lj[l, j0 + jj] = toff + jj * RPC * L
                L_lj[l, j0 + jj] = L

        # ---- scatter edges into the class streams ----
        zdst = z_of_node[dst_all]
        cls_e = lane_of_node[src_all].astype(np.int64)
        key = zdst * 4 + cls_e
        eorder = np.argsort(key, kind="stable")
        src_by = src_all[eorder]
        key_s = key[eorder]
        kstarts = np.searchsorted(key_s, np.arange(NTOT * 4))
        cnt_e = np.diff(np.concatenate([kstarts, [len(src_by)]]))
        e_key = np.repeat(np.arange(NTOT * 4), cnt_e)
        within = np.arange(len(src_by)) - np.repeat(kstarts, cnt_e)
        e_z = e_key // 4
        e_l = e_key % 4
        e_j = j_of_z[e_z]
        e_r = r_of_z[e_z]
        t_pos = base_lj[e_l, e_j] + e_r * L_lj[e_l, e_j] + within
        assert np.all(within < L_lj[e_l, e_j])
        src_w = self.word_of_z[z_of_node[src_by]].astype(np.int16)
        Tc = self.slots_per_core
        stream_idx = np.zeros((NC, GROUPS, Tc), dtype=np.int16)
        stream_idx[nc_of_z[e_z], g_of_z[e_z], t_pos] = src_w

        self.idx_wrapped = np.zeros((NC, P, Tc // RPC), dtype=np.int16)
        for nci in range(NC):
            for g in range(GROUPS):
                st = stream_idx[nci, g]
                self.idx_wrapped[nci, g * RPC:(g + 1) * RPC, :] = \
                    st.reshape(Tc // RPC, RPC).T


def _build_kernel(chunks, slots_per_core, iters=ITERS):
    from concourse import bacc, tile, mybir

    u32 = mybir.dt.uint32
    i16 = mybir.dt.int16
    Alu = mybir.AluOpType
    X = mybir.AxisListType.X

    T_core = slots_per_core
    TP = T_core // RPC
    CH = max(RPC * jr * L for (_, _, jr, L, _) in chunks)

    nc = bacc.Bacc("TRN2", target_bir_lowering=False, debug=False,
                   enable_asserts=True, num_devices=NC)
    t_idx = nc.dram_tensor("t_idx", [P, TP], i16, kind="ExternalInput")
    t_W0 = nc.dram_tensor("t_W0", [P, NWORDS], u32, kind="ExternalInput")
    t_v0 = nc.dram_tensor("t_v0", [P, NODES_CORE], u32, kind="ExternalInput")
    t_T2 = nc.dram_tensor("t_T2", [P, T2SIZE], u32, kind="ExternalInput")
    t_lane = nc.dram_tensor("t_lane", [P, RPC], u32, kind="ExternalInput")
    t_m16 = nc.dram_tensor("t_m16", [P, RPC], u32, kind="ExternalInput")
    t_cst = nc.dram_tensor("t_cst", [P, 4], u32, kind="ExternalInput")
    t_qout = nc.dram_tensor("t_qout", [P, NODES_CORE], u32, kind="ExternalOutput")

    with tile.TileContext(nc) as tc:
        with tc.tile_pool(name="dram", bufs=2, space="DRAM") as dram, \
             tc.tile_pool(name="per", bufs=1) as per, \
             tc.tile_pool(name="chk", bufs=3) as chk, \
             tc.tile_pool(name="vv", bufs=2) as vv:
            W = per.tile([P, NWORDS], u32)
            idx = per.tile([P, TP], i16)
            T2 = per.tile([P, T2SIZE], u32)
            lane = per.tile([P, RPC], u32)
            m16 = per.tile([P, RPC], u32)
            cst = per.tile([P, 4], u32)  # [8, 16, 24, 0x7]
            maskacc = per.tile([P, NODES_CORE], u32)
            maskb = per.tile([P, NODES_CORE], u32)
            classred = per.tile([P, NODES_CORE], u32)
            idxw = per.tile([P, JROWS], u32)
            idx16 = per.tile([P, JROWS], i16)
            words = per.tile([P, WORDS_CORE], u32)

            nc.sync.dma_start(out=W[:], in_=t_W0[:])
            nc.sync.dma_start(out=idx[:], in_=t_idx[:])
            nc.sync.dma_start(out=T2[:], in_=t_T2[:])
            nc.sync.dma_start(out=lane[:], in_=t_lane[:])
            nc.sync.dma_start(out=m16[:], in_=t_m16[:])
            nc.sync.dma_start(out=cst[:], in_=t_cst[:])
            v = vv.tile([P, NODES_CORE], u32, tag="v")
            nc.sync.dma_start(out=v[:], in_=t_v0[:])

            for it in range(iters):
                cur_cls = 0
                mcur, mnext = maskacc, maskb
                for (l, j0, jr, L, to) in chunks:
                    if l != cur_cls:
                        # fold finished class (class 0 wrote maskacc
                        # directly; classes 1-3 go via classred)
                        if cur_cls > 0:
                            nc.vector.scalar_tensor_tensor(
                                out=mnext[:], in0=classred[:],
                                scalar=cst[:, cur_cls - 1:cur_cls],
                                in1=mcur[:],
                                op0=Alu.logical_shift_right, op1=Alu.bitwise_or)
                            mcur, mnext = mnext, mcur
                        cur_cls = l
                    n = RPC * jr * L
                    red_out = maskacc if l == 0 else classred
                    gout = chk.tile([P, CH], u32, tag="gout")
                    nc.gpsimd.ap_gather(
                        out_ap=gout[:, :n], in_ap=W[:],
                        idxs_ap=idx[:, to // RPC:(to + n) // RPC],
                        channels=P, num_elems=NWORDS, d=1, num_idxs=n)
                    nc.vector.tensor_reduce(
                        out=red_out[:, RPC * j0:RPC * (j0 + jr)],
                        in_=gout[:, :n].rearrange("p (a b) -> p a b", b=L),
                        axis=X, op=Alu.bitwise_or)
                nc.vector.scalar_tensor_tensor(
                    out=mnext[:], in0=classred[:],
                    scalar=cst[:, cur_cls - 1:cur_cls], in1=mcur[:],
                    op0=Alu.logical_shift_right, op1=Alu.bitwise_or)
                mcur, mnext = mnext, mcur

                # T2 index: ((mask & 0xFF) << 3) | (v & 0x7)
                nc.vector.tensor_scalar(
                    out=classred[:], in0=mcur[:], scalar1=0xFF, scalar2=3,
                    op0=Alu.bitwise_and, op1=Alu.logical_shift_left)
                nc.vector.scalar_tensor_tensor(
                    out=mnext[:], in0=v[:], scalar=cst[:, 3:4],
                    in1=classred[:],
                    op0=Alu.bitwise_and, op1=Alu.bitwise_or)
                # wrapped select: idx16[p, j] = mnext[p, 16*j + p%16]
                nc.vector.tensor_tensor(
                    out=classred[:], in0=mnext[:],
                    in1=m16[:, None, :].broadcast_to([P, JROWS, RPC]),
                    op=Alu.bitwise_and)
                nc.vector.tensor_reduce(
                    out=idxw[:],
                    in_=classred[:].rearrange("p (a b) -> p a b", b=RPC),
                    axis=X, op=Alu.bitwise_or)
                nc.vector.tensor_copy(idx16[:], idxw[:])
                vn = vv.tile([P, NODES_CORE], u32, tag="v")
                nc.gpsimd.ap_gather(out_ap=vn[:], in_ap=T2[:],
                                    idxs_ap=idx16[:], channels=P,
                                    num_elems=T2SIZE, d=1,
                                    num_idxs=NODES_CORE)
                if it < iters - 1:
                    # repack words: ((v >> 16) << lanecode), OR over 4 lanes
                    nc.vector.scalar_tensor_tensor(
                        out=classred[:].rearrange("p (a b) -> p a b", b=RPC),
                        in0=vn[:].rearrange("p (a b) -> p a b", b=RPC),
                        scalar=cst[:, 1:2],
                        in1=lane[:, None, :].broadcast_to([P, JROWS, RPC]),
                        op0=Alu.logical_shift_right, op1=Alu.logical_shift_left)
                    nc.vector.tensor_reduce(
                        out=words[:],
                        in_=classred[:].rearrange("p (a b) -> p a b", b=4),
                        axis=X, op=Alu.bitwise_or)
                    dwords = dram.tile([1, WORDS_NC], u32, tag="dw")
                    dgath = dram.tile([1, NC * WORDS_NC], u32, tag="dg")
                    nc.sync.dma_start(out=dwords[:], in_=words[0::RPC, :])
                    nc.gpsimd.collective_compute(
                        "AllGather", Alu.bypass,
                        replica_groups=[list(range(NC))],
                        ins=[dwords.opt()], outs=[dgath.opt()])
                    nc.sync.dma_start(
                        out=W[:, 1:],
                        in_=dgath[0:1, :].broadcast_to([P, NC * WORDS_NC]))
                v = vn
            nc.sync.dma_start(out=t_qout[:], in_=v[:])
    nc.compile()
    return nc


def _device_inputs(lay, s0, T):
    ns_tab = np.argmax(np.asarray(T), axis=2).astype(np.uint32)  # [256, 8]
    T2 = np.zeros(T2SIZE, dtype=np.uint32)
    # idx = (mask << 3) | state; value = (1<<next)<<16 | next
    flat = ns_tab.reshape(-1)  # mask*8 + state
    T2[:] = ((np.uint32(1) << flat) << 16) | flat

    st_node = np.argmax(np.asarray(s0), axis=1).astype(np.uint32)
    st_z = np.zeros(NTOT, dtype=np.uint32)
    valid = lay.node_of_z >= 0
    st_z[valid] = st_node[lay.node_of_z[valid]]
    W0 = np.zeros(NWORDS, dtype=np.uint32)
    byte = (np.uint32(1) << st_z) << (8 * lay.lane_of_z)
    np.bitwise_or.at(W0, lay.word_of_z, byte)

    v0_z = ((np.uint32(1) << st_z) << 16) | st_z

    stg = np.zeros((NC, P, JROWS), dtype=np.uint32)
    stg[lay.nc_of_z, lay.p_of_z, lay.j_of_z] = v0_z
    lanecode = np.broadcast_to(((np.arange(RPC) % 4) * 8).astype(np.uint32),
                               (P, RPC)).copy()
    m16 = np.zeros((P, RPC), dtype=np.uint32)
    m16[np.arange(P), np.arange(P) % RPC] = 0xFFFFFFFF
    cst = np.broadcast_to(np.array([8, 16, 24, 0x7], dtype=np.uint32),
                          (P, 4)).copy()
    W0b = np.broadcast_to(W0, (P, NWORDS)).copy()
    T2b = np.broadcast_to(T2, (P, T2SIZE)).copy()

    in_maps = []
    for nci in range(NC):
        v0 = np.zeros((P, NODES_CORE), dtype=np.uint32)
        for g in range(GROUPS):
            sgrid = stg[nci, g * RPC:(g + 1) * RPC, :]  # [16, 98]
            v0[g * RPC:(g + 1) * RPC, :] = sgrid.T.reshape(-1)[None, :]
        in_maps.append({
            "t_idx": lay.idx_wrapped[nci],
            "t_W0": W0b,
            "t_v0": v0,
            "t_T2": T2b,
            "t_lane": lanecode,
            "t_m16": m16,
            "t_cst": cst,
        })
    return in_maps


def _decode(lay, results):
    stg = np.zeros((NC, P, JROWS), dtype=np.uint32)
    for nci in range(NC):
        qout = results[nci]["t_qout"]
        for g in range(GROUPS):
            stg[nci, g * RPC:(g + 1) * RPC, :] = \
                qout[g * RPC, :].reshape(JROWS, RPC).T
    valid = lay.node_of_z >= 0
    st_z = stg[lay.nc_of_z[valid], lay.p_of_z[valid],
               lay.j_of_z[valid]].astype(np.int64) & 0x7
    st_node = np.zeros(N_REAL, dtype=np.int64)
    st_node[lay.node_of_z[valid]] = st_z
    out = np.zeros((N_REAL, S), dtype=np.float32)
    out[np.arange(N_REAL), st_node] = 1.0
    return out


def kernel(s0, edge_index, T):
    global LAST_EXEC_NS, LAST_TRACE_PATH
    from concourse import bass_utils

    s0 = np.asarray(s0)
    edge_index = np.asarray(edge_index)
    Tn = np.asarray(T)
    lay = _Layout(edge_index)
    nc = _build_kernel(lay.chunks, lay.slots_per_core)
    in_maps = _device_inputs(lay, s0, Tn)
    trace = os.environ.get("BASS_FSM_TRACE", "0") == "1"
    res = bass_utils.run_bass_kernel_spmd(
        nc, in_maps, core_ids=list(range(NC)), trace=trace)
    LAST_EXEC_NS = res.exec_time_ns
    if res.instructions_and_trace is not None:
        LAST_TRACE_PATH = res.instructions_and_trace[1]
    return _decode(lay, res.results).astype(s0.dtype)


# revision 11
# speedup vs baseline: 1.5539x; 1.5539x over previous
"""Self-contained TRN2 Bass kernel for NeuralFSM message passing.

kernel(s0, edge_index, T) -> [100000, 8] float32, running 20 FSM iterations
on 8 NeuronCores via concourse/bass (SPMD, node-partitioned).

Algorithm: states are one-hot; threshold(segment_sum) == bitwise OR of
neighbor state bits. Per node keep a byte b = 1<<state packed 4-per-uint32
in an SBUF-resident table replicated across partitions; per iteration
ap_gather the static edge-slot streams, shift-extract the source byte,
OR-reduce per destination (uniform padded slot count per chunk), look up the
256x8 FSM transition via two small gathers, AllGather the rebuilt table
words across the 8 cores, and re-broadcast.
"""
import os
import sys

import numpy as np

for _p in ("/opt/trn_rl_repo", "/root/.axon_site/_ro/trn_rl_repo", "/root/.axon_site"):
    if os.path.isdir(_p) and _p not in sys.path:
        sys.path.append(_p)

N_REAL = 100000
S = 8
NC = 8
P = 128
CORES = 8
RPC = 16
JROWS = 98
NODES_CORE = RPC * JROWS          # 1568
NDST_NC = P * JROWS               # 12544
NTOT = NC * NDST_NC               # 100352
WORDS_CORE = NODES_CORE // 4      # 392
WORDS_NC = WORDS_CORE * CORES     # 3136
NWORDS = 1 + NC * WORDS_NC        # 25089
T2SIZE = 2049
ITERS = 20
CHUNK_BUDGET = 4096

LAST_EXEC_NS = None
LAST_TRACE_PATH = None


class _Layout:
    def __init__(self, edge_index):
        src_all = edge_index[0].astype(np.int64)
        dst_all = edge_index[1].astype(np.int64)
        deg = np.bincount(dst_all, minlength=N_REAL)
        L_node = np.maximum(1, -(-deg // 4)) * 4
        order = np.argsort(-L_node, kind="stable")
        node_of_z = np.full(NTOT, -1, dtype=np.int64)
        node_of_z[:N_REAL] = order
        z = np.arange(NTOT)
        self.j_of_z = z // (NC * P)
        self.nc_of_z = (z % (NC * P)) // P
        self.p_of_z = z % P
        self.node_of_z = node_of_z
        z_of_node = np.full(N_REAL, -1, dtype=np.int64)
        z_of_node[order] = np.arange(N_REAL)
        self.z_of_node = z_of_node

        Lz = np.zeros(NTOT, dtype=np.int64)
        Lz[:N_REAL] = L_node[order]
        self.L_row = np.maximum(1, Lz.reshape(JROWS, NC * P).max(axis=1) // 4) * 4

        c_of_z = self.p_of_z // RPC
        r_of_z = self.p_of_z % RPC
        self.word_of_z = 1 + self.nc_of_z * WORDS_NC + c_of_z * WORDS_CORE \
            + 4 * self.j_of_z + r_of_z // 4
        self.lane_of_z = r_of_z % 4

        chunks = []
        j = 0
        while j < JROWS:
            j2 = j
            curmax = 0
            while j2 < JROWS:
                m2 = max(curmax, int(self.L_row[j2]))
                if RPC * (j2 - j + 1) * m2 > CHUNK_BUDGET and j2 > j:
                    break
                curmax = m2
                j2 += 1
                if RPC * m2 > CHUNK_BUDGET:
                    break
            chunks.append((j, j2 - j, curmax))
            j = j2
        self.chunks = chunks
        self.slots_per_core = int(sum(RPC * jr * L for (_, jr, L) in chunks))

        # edges grouped by dst placement
        zdst = z_of_node[dst_all]
        eorder = np.argsort(zdst, kind="stable")
        src_by_z = src_all[eorder]
        zsorted = zdst[eorder]
        starts = np.searchsorted(zsorted, np.arange(NTOT))
        ends = np.searchsorted(zsorted, np.arange(NTOT) + 1)
        src_w = self.word_of_z[z_of_node[src_by_z]].astype(np.int16)
        src_sh = (8 * self.lane_of_z[z_of_node[src_by_z]]).astype(np.uint8)

        Tc = self.slots_per_core
        # stream position of slot s of dst z: per (nc,c): t = chunk_off + ((jj*16+r)*L) + s
        # build per-z slot base in stream, then scatter srcs
        row_off = np.zeros(JROWS, dtype=np.int64)     # chunk_stream_off + jj*16*L
        row_L = np.zeros(JROWS, dtype=np.int64)
        to = 0
        for (j0, jr, L) in chunks:
            for jj in range(jr):
                row_off[j0 + jj] = to + jj * RPC * L
                row_L[j0 + jj] = L
            to += RPC * jr * L
        base_z = row_off[self.j_of_z] + (r_of_z) * row_L[self.j_of_z]
        # expand: slot position for each sorted edge
        cnt = ends - starts
        e_z = np.repeat(np.arange(NTOT), cnt)
        within = np.arange(len(src_by_z)) - np.repeat(starts, cnt)
        t_pos = base_z[e_z] + within
        stream_idx = np.zeros((NC, CORES, Tc), dtype=np.int16)
        stream_sh = np.zeros((NC, CORES, Tc), dtype=np.uint8)
        stream_idx[self.nc_of_z[e_z], c_of_z[e_z], t_pos] = src_w
        stream_sh[self.nc_of_z[e_z], c_of_z[e_z], t_pos] = src_sh
        self.stream_sh = stream_sh

        self.idx_wrapped = np.zeros((NC, P, Tc // RPC), dtype=np.int16)
        for nc_ in range(NC):
            for c in range(CORES):
                st = stream_idx[nc_, c]
                self.idx_wrapped[nc_, c * RPC:(c + 1) * RPC, :] = \
                    st.reshape(Tc // RPC, RPC).T


def _build_kernel(chunks, slots_per_core, iters=ITERS):
    from concourse import bacc, tile, mybir

    u32 = mybir.dt.uint32
    u8 = mybir.dt.uint8
    i16 = mybir.dt.int16
    Alu = mybir.AluOpType
    X = mybir.AxisListType.X

    T_core = slots_per_core
    TP = T_core // 16
    CH = max(RPC * jr * L for (_, jr, L) in chunks)

    nc = bacc.Bacc("TRN2", target_bir_lowering=False, debug=False,
                   enable_asserts=True, num_devices=NC)
    t_idx = nc.dram_tensor("t_idx", [P, TP], i16, kind="ExternalInput")
    t_shift = nc.dram_tensor("t_shift", [P, T_core], u8, kind="ExternalInput")
    t_W0 = nc.dram_tensor("t_W0", [P, NWORDS], u32, kind="ExternalInput")
    t_q0 = nc.dram_tensor("t_q0", [P, NODES_CORE], u32, kind="ExternalInput")
    t_T2N1 = nc.dram_tensor("t_T2N1", [P, T2SIZE], u32, kind="ExternalInput")
    t_T2L0 = nc.dram_tensor("t_T2L0", [P, T2SIZE], u32, kind="ExternalInput")
    t_lane = nc.dram_tensor("t_lane", [P, RPC], u32, kind="ExternalInput")
    t_m16 = nc.dram_tensor("t_m16", [P, RPC], u32, kind="ExternalInput")
    t_qout = nc.dram_tensor("t_qout", [P, NODES_CORE], u32, kind="ExternalOutput")

    with tile.TileContext(nc) as tc:
        with tc.tile_pool(name="dram", bufs=2, space="DRAM") as dram, \
             tc.tile_pool(name="per", bufs=1) as per, \
             tc.tile_pool(name="chk", bufs=2) as chk, \
             tc.tile_pool(name="sh32", bufs=1) as sh32p, \
             tc.tile_pool(name="qq", bufs=2) as qq:
            W = per.tile([P, NWORDS], u32)
            idx = per.tile([P, TP], i16)
            T2N1 = per.tile([P, T2SIZE], u32)
            T2L0 = per.tile([P, T2SIZE], u32)
            lane = per.tile([P, RPC], u32)
            mask = per.tile([P, NODES_CORE], u32)
            tmp = per.tile([P, NODES_CORE + 16], u32)  # +16: strided-read footprint guard
            idx16 = per.tile([P, JROWS], i16)
            words = per.tile([P, WORDS_CORE], u32)
            m16 = per.tile([P, RPC], u32)
            idxw = per.tile([P, JROWS], u32)

            nc.gpsimd.memset(tmp[:], 0)
            nc.sync.dma_start(out=W[:], in_=t_W0[:])
            nc.sync.dma_start(out=idx[:], in_=t_idx[:])
            nc.sync.dma_start(out=T2N1[:], in_=t_T2N1[:])
            nc.sync.dma_start(out=T2L0[:], in_=t_T2L0[:])
            nc.sync.dma_start(out=lane[:], in_=t_lane[:])
            nc.sync.dma_start(out=m16[:], in_=t_m16[:])
            q = qq.tile([P, NODES_CORE], u32, tag="q")
            nc.sync.dma_start(out=q[:], in_=t_q0[:])

            for it in range(iters):
                mo = 0
                to = 0
                for (j0, jr, L) in chunks:
                    n = RPC * jr * L
                    jr16 = RPC * jr
                    gout = chk.tile([P, CH], u32, tag="gout")
                    nc.gpsimd.ap_gather(
                        out_ap=gout[:, :n], in_ap=W[:],
                        idxs_ap=idx[:, to // 16:(to + n) // 16],
                        channels=P, num_elems=NWORDS, d=1, num_idxs=n)
                    shu8 = chk.tile([P, CH], u8, tag="shu8")
                    nc.sync.dma_start(out=shu8[:, :n], in_=t_shift[:, to:to + n])
                    shu32 = sh32p.tile([P, (CH + 1) // 2], u32, tag="shu32")
                    h = (n + 63) // 128 * 64
                    for s0, s1 in ((0, min(h, n)), (min(h, n), n)):
                        if s1 <= s0:
                            continue
                        nc.vector.tensor_copy(shu32[:, :s1 - s0],
                                              shu8[:, s0:s1])
                        nc.vector.tensor_tensor(
                            out=gout[:, s0:s1], in0=gout[:, s0:s1],
                            in1=shu32[:, :s1 - s0],
                            op=Alu.logical_shift_right)
                    nc.vector.tensor_reduce(
                        out=mask[:, mo:mo + jr16],
                        in_=gout[:, :n].rearrange("p (a b) -> p a b", b=L),
                        axis=X, op=Alu.bitwise_or)
                    mo += jr16
                    to += n
                assert mo == NODES_CORE and to == T_core

                nc.vector.tensor_scalar(
                    out=tmp[:, :NODES_CORE], in0=mask[:], scalar1=0xFF, scalar2=3,
                    op0=Alu.bitwise_and, op1=Alu.logical_shift_left)
                nc.vector.tensor_tensor(out=tmp[:, :NODES_CORE], in0=tmp[:, :NODES_CORE], in1=q[:],
                                        op=Alu.add)
                # wrapped select: idx16[p, j] = tmp[p, 16*j + p%16]
                nc.vector.tensor_tensor(
                    out=tmp[:, :NODES_CORE], in0=tmp[:, :NODES_CORE],
                    in1=m16[:, None, :].broadcast_to([P, JROWS, RPC]),
                    op=Alu.bitwise_and)
                nc.vector.tensor_reduce(
                    out=idxw[:],
                    in_=tmp[:, :NODES_CORE].rearrange("p (a b) -> p a b", b=RPC),
                    axis=X, op=Alu.bitwise_or)
                nc.vector.tensor_copy(idx16[:], idxw[:])
                qn = qq.tile([P, NODES_CORE], u32, tag="q")
                if it == iters - 1:
                    nc.gpsimd.ap_gather(out_ap=qn[:], in_ap=T2N1[:],
                                        idxs_ap=idx16[:], channels=P,
                                        num_elems=T2SIZE, d=1,
                                        num_idxs=NODES_CORE)
                q = qn
                if it < iters - 1:
                    nc.gpsimd.ap_gather(out_ap=tmp[:, :NODES_CORE], in_ap=T2L0[:],
                                        idxs_ap=idx16[:], channels=P,
                                        num_elems=T2SIZE, d=1,
                                        num_idxs=NODES_CORE)
                    nc.vector.tensor_tensor(
                        out=tmp[:, :NODES_CORE].rearrange("p (a b) -> p a b", b=RPC),
                        in0=tmp[:, :NODES_CORE].rearrange("p (a b) -> p a b", b=RPC),
                        in1=lane[:, None, :].broadcast_to([P, JROWS, RPC]),
                        op=Alu.logical_shift_left)
                    nc.vector.tensor_reduce(
                        out=words[:], in_=tmp[:, :NODES_CORE].rearrange("p (a b) -> p a b", b=4),
                        axis=X, op=Alu.bitwise_or)
                    dwords = dram.tile([1, WORDS_NC], u32, tag="dw")
                    dgath = dram.tile([1, NC * WORDS_NC], u32, tag="dg")
                    nc.sync.dma_start(out=dwords[:], in_=words[0::16, :])
                    nc.gpsimd.collective_compute(
                        "AllGather", Alu.bypass,
                        replica_groups=[list(range(NC))],
                        ins=[dwords.opt()], outs=[dgath.opt()])
                    # T2N gather emitted after the collective trigger so it
                    # overlaps the AllGather on the TOPSP engines
                    nc.gpsimd.ap_gather(out_ap=qn[:], in_ap=T2N1[:],
                                        idxs_ap=idx16[:], channels=P,
                                        num_elems=T2SIZE, d=1,
                                        num_idxs=NODES_CORE)
                    nc.sync.dma_start(
                        out=W[:, 1:],
                        in_=dgath[0:1, :].broadcast_to([P, NC * WORDS_NC]))
            nc.sync.dma_start(out=t_qout[:], in_=q[:])
    nc.compile()
    return nc


def _device_inputs(lay, s0, T):
    ns_tab = np.argmax(np.asarray(T), axis=2).astype(np.uint32)  # [256, 8]
    flat = ns_tab.reshape(-1)  # idx-1 = mask*8 + state
    T2N1 = np.zeros(T2SIZE, dtype=np.uint32)
    T2N1[1:] = flat + 1
    T2L0 = np.zeros(T2SIZE, dtype=np.uint32)
    T2L0[1:] = (1 << flat).astype(np.uint32)

    st_node = np.argmax(np.asarray(s0), axis=1).astype(np.uint32)
    st_z = np.zeros(NTOT, dtype=np.uint32)
    st_z[:N_REAL] = st_node[lay.node_of_z[:N_REAL]]
    W0 = np.zeros(NWORDS, dtype=np.uint32)
    byte = (1 << st_z).astype(np.uint32) << (8 * lay.lane_of_z)
    np.bitwise_or.at(W0, lay.word_of_z, byte)

    stg = np.zeros((NC, P, JROWS), dtype=np.uint32)
    stg[lay.nc_of_z, lay.p_of_z, lay.j_of_z] = st_z
    lanecode = np.broadcast_to(((np.arange(RPC) % 4) * 8).astype(np.uint32), (P, RPC)).copy()
    m16 = np.zeros((P, RPC), dtype=np.uint32)
    m16[np.arange(P), np.arange(P) % RPC] = 0xFFFFFFFF
    W0b = np.broadcast_to(W0, (P, NWORDS)).copy()
    T2N1b = np.broadcast_to(T2N1, (P, T2SIZE)).copy()
    T2L0b = np.broadcast_to(T2L0, (P, T2SIZE)).copy()

    in_maps = []
    for nci in range(NC):
        q0 = np.zeros((P, NODES_CORE), dtype=np.uint32)
        shf = np.zeros((P, lay.slots_per_core), dtype=np.uint8)
        for c in range(CORES):
            sgrid = stg[nci, c * RPC:(c + 1) * RPC, :]
            q0[c * RPC:(c + 1) * RPC, :] = (sgrid.T.reshape(-1) + 1)[None, :]
            shf[c * RPC:(c + 1) * RPC, :] = lay.stream_sh[nci, c][None, :]
        in_maps.append({
            "t_idx": lay.idx_wrapped[nci],
            "t_shift": shf,
            "t_W0": W0b,
            "t_q0": q0,
            "t_T2N1": T2N1b,
            "t_T2L0": T2L0b,
            "t_lane": lanecode,
            "t_m16": m16,
        })
    return in_maps


def _decode(lay, results):
    stg = np.zeros((NC, P, JROWS), dtype=np.uint32)
    for nci in range(NC):
        qout = results[nci]["t_qout"]
        for c in range(CORES):
            stg[nci, c * RPC:(c + 1) * RPC, :] = \
                qout[c * RPC, :].reshape(JROWS, RPC).T
    st_z = stg[lay.nc_of_z[:N_REAL], lay.p_of_z[:N_REAL],
               lay.j_of_z[:N_REAL]].astype(np.int64) - 1
    st_node = np.zeros(N_REAL, dtype=np.int64)
    st_node[lay.node_of_z[:N_REAL]] = st_z
    out = np.zeros((N_REAL, S), dtype=np.float32)
    out[np.arange(N_REAL), st_node] = 1.0
    return out


def kernel(s0, edge_index, T):
    global LAST_EXEC_NS, LAST_TRACE_PATH
    from concourse import bass_utils

    s0 = np.asarray(s0)
    edge_index = np.asarray(edge_index)
    Tn = np.asarray(T)
    lay = _Layout(edge_index)
    nc = _build_kernel(lay.chunks, lay.slots_per_core)
    in_maps = _device_inputs(lay, s0, Tn)
    trace = os.environ.get("BASS_FSM_TRACE", "0") == "1"
    res = bass_utils.run_bass_kernel_spmd(
        nc, in_maps, core_ids=list(range(NC)), trace=trace)
    LAST_EXEC_NS = res.exec_time_ns
    if res.instructions_and_trace is not None:
        LAST_TRACE_PATH = res.instructions_and_trace[1]
    return _decode(lay, res.results).astype(s0.dtype)

